# revision 10
# baseline (speedup 1.0000x reference)
"""Trainium2 Bass kernel for 2-layer LSTM token sampling (nn_ORGAN_67834713473538).

Strategy: data-parallel over the batch (1024 seqs -> 128 per core on 8 cores).
All state kept batch-major on chip; per step:
  g0 = onehot @ [emb@w_ih_0.T ; b0]  +  hA @ w_hh_0.T          (f32 matmuls)
  hA,cA = lstm_act(g0); hT_A = transpose(hA)
  g1 = b1(bf16 triple)  +  hA @ w_ih_1.T  +  hB @ w_hh_1.T
  hB,cB = lstm_act(g1); hT_B = transpose(hB)
  logits = hB @ w_out.T (+ b_out)
  sampled = argmax(logits + gumbel[t])        (gumbel precomputed on host,
                                               = jax.random.categorical exactly)
  current = where(is_end, PAD, sampled); lengths += ~is_end; is_end |= current==EOS
"""
import sys, os
for _p in ('/opt/trn_rl_repo', '/root/.axon_site/_ro/trn_rl_repo'):
    if os.path.isdir(_p) and _p not in sys.path:
        sys.path.insert(0, _p)

import numpy as np
from concourse import bass, tile, mybir

dt = mybir.dt

VOCAB, EMBED, HIDDEN, LAYERS = 30, 32, 512, 2
PAD, BOS, EOS = 0, 1, 2
B_FULL, MAX_LEN = 1024, 64
N_CORES = 8
B = B_FULL // N_CORES          # 128 per core
H4 = 4 * HIDDEN                # 2048
KT = HIDDEN // 128             # 4 k-tiles
NCH = H4 // 512                # 4 gate chunks of 512


# ---------------------------------------------------------------- wait fixup
def split_multiwaits(nc):
    """This walrus build accepts only one sync wait per instruction; hoist
    extra waits onto injected same-engine NoOps placed just before."""
    n = 0
    for f in nc.m.functions:
        for bb in f.blocks:
            out = []
            changed = False
            for ins in bb.instructions:
                si = ins.sync_info
                if si is not None and si.on_wait and len(si.on_wait) > 1:
                    ws = list(si.on_wait)
                    for w in ws[:-1]:
                        nop = mybir.InstNoOp(
                            name=f"wsplit-{nc.next_id()}",
                            engine=ins.engine,
                            sync_info=mybir.SyncInfo(on_wait=[w], on_update=[]),
                        )
                        out.append(nop)
                        n += 1
                    si.on_wait = [ws[-1]]
                    changed = True
                out.append(ins)
            if changed:
                bb.instructions = out
    return n


# ---------------------------------------------------------------- blob layout
def _blob_layout(T):
    lay, off = {}, 0
    for name, w in [("ident", 128), ("w0e", H4), ("whh0", KT * H4),
                    ("wih1", KT * H4), ("whh1", KT * H4), ("wout", KT * VOCAB),
                    ("b13", H4), ("gumb", T * VOCAB), ("bout", VOCAB),
                    ("iota", 1), ("prev", 1)]:
        lay[name] = (off, off + w)
        off += w
    return lay, off


# ---------------------------------------------------------------- device code
def build_nc(T):
    nc = bass.Bass(target_bir_lowering=False, trn_type="TRN2")
    f32, bf16, i32, u32 = dt.float32, dt.bfloat16, dt.int32, dt.uint32
    AF = mybir.ActivationFunctionType
    ALU = mybir.AluOpType

    # single packed input blob [128, ncols]; layout mirrored in _blob_layout(T)
    lay, ncols = _blob_layout(T)
    D_blob = nc.dram_tensor("blob", [128, ncols], f32, kind="ExternalInput")

    def bl(name):
        a, b = lay[name]
        return D_blob[:, a:b]

    D_seq = nc.dram_tensor("seq_o", [B, T], i32, kind="ExternalOutput")
    D_log = nc.dram_tensor("logits_o", [B, T * VOCAB], f32, kind="ExternalOutput")
    D_len = nc.dram_tensor("len_o", [B, 1], i32, kind="ExternalOutput")

    with tile.TileContext(nc) as tc:
        with tc.tile_pool(name="const", bufs=1) as cp, \
             tc.tile_pool(name="state", bufs=1) as st, \
             tc.tile_pool(name="scratch", bufs=2) as sc, \
             tc.tile_pool(name="psA", bufs=4, space="PSUM") as psA, \
             tc.tile_pool(name="psB", bufs=4, space="PSUM") as psB:

            # ---- constants / weights ----
            w0e = cp.tile([VOCAB + 1, H4], f32)
            nc.sync.dma_start(w0e[:], bl("w0e")[0:VOCAB + 1, :])
            whh0 = cp.tile([128, KT, H4], f32)
            wih1 = cp.tile([128, KT, H4], f32)
            whh1 = cp.tile([128, KT, H4], f32)
            wout = cp.tile([128, KT, VOCAB], f32)
            for kt in range(KT):
                a0, _ = lay["whh0"]
                nc.sync.dma_start(whh0[:, kt, :],
                                  D_blob[:, a0 + kt * H4:a0 + (kt + 1) * H4])
                a0, _ = lay["wih1"]
                nc.sync.dma_start(wih1[:, kt, :],
                                  D_blob[:, a0 + kt * H4:a0 + (kt + 1) * H4])
                a0, _ = lay["whh1"]
                nc.sync.dma_start(whh1[:, kt, :],
                                  D_blob[:, a0 + kt * H4:a0 + (kt + 1) * H4])
                a0, _ = lay["wout"]
                nc.sync.dma_start(wout[:, kt, :],
                                  D_blob[:, a0 + kt * VOCAB:a0 + (kt + 1) * VOCAB])
            b13f = cp.tile([3, H4], f32)
            nc.sync.dma_start(b13f[:], bl("b13")[0:3, :])
            b13 = cp.tile([3, H4], bf16)
            nc.vector.tensor_copy(b13[:], b13f[:])
            ones3 = cp.tile([3, 128], bf16)
            nc.vector.memset(ones3[:], 1.0)
            ones31 = cp.tile([1, VOCAB + 1], f32)
            nc.vector.memset(ones31[:], 1.0)
            gumb = cp.tile([B, T * VOCAB], f32)
            nc.sync.dma_start(gumb[:], bl("gumb"))
            bout = cp.tile([B, VOCAB], f32)
            nc.sync.dma_start(bout[:], bl("bout"))
            ident = cp.tile([128, 128], f32)
            nc.sync.dma_start(ident[:], bl("ident"))
            iota30 = cp.tile([VOCAB, 1], f32)
            nc.sync.dma_start(iota30[:], bl("iota")[0:VOCAB, :])

            # ---- state ----
            cA = st.tile([B, HIDDEN], f32)
            cB = st.tile([B, HIDDEN], f32)
            hTA = st.tile([128, KT, 128], f32)
            hTB = st.tile([128, KT, 128], f32)
            nc.vector.memset(cA[:], 0.0)
            nc.vector.memset(cB[:], 0.0)
            nc.vector.memset(hTA[:], 0.0)
            nc.vector.memset(hTB[:], 0.0)
            current = st.tile([B, 1], f32)
            is_end = st.tile([B, 1], f32)
            lengths = st.tile([B, 1], f32)
            nc.vector.memset(lengths[:], 0.0)
            nc.sync.dma_start(current[:], bl("prev"))
            nc.vector.tensor_scalar(is_end[:], current[:], float(EOS), None,
                                    ALU.is_equal)
            onehot = st.tile([VOCAB + 1, 128], f32)
            nc.vector.memset(onehot[:], 1.0)   # row 30 stays 1.0 forever
            seq_f = st.tile([B, T], f32)
            out_log = st.tile([B, T * VOCAB], f32)

            for t in range(T):
                # ---- onehot(current) ----
                curT = psB.tile([1, 128], f32, tag="psB", name=f"curT{t}")
                nc.tensor.transpose(curT[:], current[:], ident[:])
                cur_row = sc.tile([1, 128], f32, tag="cur_row", name=f"cur_row{t}")
                nc.vector.tensor_copy(cur_row[:], curT[:])
                bc = psB.tile([VOCAB + 1, 128], f32, tag="psB", name=f"bc{t}")
                nc.tensor.matmul(bc[:], ones31[:], cur_row[:], start=True, stop=True)
                nc.vector.tensor_scalar(onehot[0:VOCAB, :], bc[0:VOCAB, :],
                                        iota30[:], None, ALU.is_equal)

                # ---- layer 0 gates ----
                g0c = []
                for c in range(NCH):
                    g = psA.tile([B, 512], f32, tag="psA", name=f"g0_{t}_{c}")
                    g0c.append(g)
                    cs = slice(c * 512, (c + 1) * 512)
                    nc.tensor.matmul(g[:], onehot[:], w0e[:, cs], start=True, stop=False)
                    for kt in range(KT):
                        nc.tensor.matmul(g[:], hTA[:, kt, :], whh0[:, kt, cs],
                                         start=False, stop=(kt == KT - 1))

                # ---- layer 0 activations ----
                sig_i = sc.tile([B, 512], f32, tag="sig_i", name=f"sig_i{t}")
                sig_f = sc.tile([B, 512], f32, tag="sig_f", name=f"sig_f{t}")
                tnh_g = sc.tile([B, 512], f32, tag="tnh_g", name=f"tnh_g{t}")
                sig_o = sc.tile([B, 512], f32, tag="sig_o", name=f"sig_o{t}")
                nc.scalar.activation(sig_i[:], g0c[0][:], AF.Sigmoid)
                nc.scalar.activation(sig_f[:], g0c[1][:], AF.Sigmoid)
                nc.scalar.activation(tnh_g[:], g0c[2][:], AF.Tanh)
                nc.scalar.activation(sig_o[:], g0c[3][:], AF.Sigmoid)
                t1 = sc.tile([B, 512], f32, tag="t1", name=f"t1_{t}")
                t2 = sc.tile([B, 512], f32, tag="t2", name=f"t2_{t}")
                nc.vector.tensor_mul(t1[:], sig_i[:], tnh_g[:])
                nc.vector.tensor_mul(t2[:], sig_f[:], cA[:])
                nc.vector.tensor_add(cA[:], t1[:], t2[:])
                tnh_c = sc.tile([B, 512], f32, tag="tnh_c", name=f"tnh_c{t}")
                nc.scalar.activation(tnh_c[:], cA[:], AF.Tanh)
                hA = sc.tile([B, 512], f32, tag="hA", name=f"hA{t}")
                nc.vector.tensor_mul(hA[:], sig_o[:], tnh_c[:])

                # ---- transpose hA -> hTA ----
                for kt in range(KT):
                    tp = psA.tile([128, 128], f32, tag="psA", name=f"tpa{t}_{kt}")
                    nc.tensor.transpose(tp[:], hA[:, kt * 128:(kt + 1) * 128], ident[:])
                    nc.vector.tensor_copy(hTA[:, kt, :], tp[:])

                # ---- layer 1 gates ----
                g1c = []
                for c in range(NCH):
                    g = psB.tile([B, 512], f32, tag="psB", name=f"g1_{t}_{c}")
                    g1c.append(g)
                    cs = slice(c * 512, (c + 1) * 512)
                    nc.tensor.matmul(g[:], ones3[:], b13[:, cs], start=True, stop=False)
                    for kt in range(KT):
                        nc.tensor.matmul(g[:], hTA[:, kt, :], wih1[:, kt, cs],
                                         start=False, stop=False)
                    for kt in range(KT):
                        nc.tensor.matmul(g[:], hTB[:, kt, :], whh1[:, kt, cs],
                                         start=False, stop=(kt == KT - 1))

                # ---- layer 1 activations ----
                sig_i1 = sc.tile([B, 512], f32, tag="sig_i", name=f"sig_i1_{t}")
                sig_f1 = sc.tile([B, 512], f32, tag="sig_f", name=f"sig_f1_{t}")
                tnh_g1 = sc.tile([B, 512], f32, tag="tnh_g", name=f"tnh_g1_{t}")
                sig_o1 = sc.tile([B, 512], f32, tag="sig_o", name=f"sig_o1_{t}")
                nc.scalar.activation(sig_i1[:], g1c[0][:], AF.Sigmoid)
                nc.scalar.activation(sig_f1[:], g1c[1][:], AF.Sigmoid)
                nc.scalar.activation(tnh_g1[:], g1c[2][:], AF.Tanh)
                nc.scalar.activation(sig_o1[:], g1c[3][:], AF.Sigmoid)
                t11 = sc.tile([B, 512], f32, tag="t1", name=f"t11_{t}")
                t21 = sc.tile([B, 512], f32, tag="t2", name=f"t21_{t}")
                nc.vector.tensor_mul(t11[:], sig_i1[:], tnh_g1[:])
                nc.vector.tensor_mul(t21[:], sig_f1[:], cB[:])
                nc.vector.tensor_add(cB[:], t11[:], t21[:])
                tnh_c1 = sc.tile([B, 512], f32, tag="tnh_c", name=f"tnh_c1_{t}")
                nc.scalar.activation(tnh_c1[:], cB[:], AF.Tanh)
                hB = sc.tile([B, 512], f32, tag="hB", name=f"hB{t}")
                nc.vector.tensor_mul(hB[:], sig_o1[:], tnh_c1[:])

                # ---- transpose hB -> hTB ----
                for kt in range(KT):
                    tp = psB.tile([128, 128], f32, tag="psB", name=f"tpb{t}_{kt}")
                    nc.tensor.transpose(tp[:], hB[:, kt * 128:(kt + 1) * 128], ident[:])
                    nc.vector.tensor_copy(hTB[:, kt, :], tp[:])

                # ---- logits ----
                L = psB.tile([B, VOCAB], f32, tag="psB", name=f"L{t}")
                for kt in range(KT):
                    nc.tensor.matmul(L[:], hTB[:, kt, :], wout[:, kt, :],
                                     start=(kt == 0), stop=(kt == KT - 1))
                ts30 = slice(t * VOCAB, (t + 1) * VOCAB)
                nc.vector.tensor_add(out_log[:, ts30], L[:], bout[:])
                scores = sc.tile([B, VOCAB], f32, tag="scores", name=f"scores{t}")
                nc.vector.tensor_add(scores[:], L[:], gumb[:, ts30])

                # ---- sample: argmax(scores) ----
                mx8 = sc.tile([B, 8], f32, tag="mx8", name=f"mx8_{t}")
                idx8 = sc.tile([B, 8], u32, tag="idx8", name=f"idx8_{t}")
                nc.vector.max(mx8[:], scores[:])
                nc.vector.max_index(idx8[:], mx8[:], scores[:])
                samp = sc.tile([B, 1], f32, tag="samp", name=f"samp{t}")
                nc.vector.tensor_copy(samp[:], idx8[:, 0:1])

                # nie = (is_end == 0);  current = samp * nie  (PAD == 0)
                nie = sc.tile([B, 1], f32, tag="nie", name=f"nie{t}")
                nc.vector.tensor_scalar(nie[:], is_end[:], 0.0, None, ALU.is_equal)
                nc.vector.tensor_mul(current[:], samp[:], nie[:])
                nc.vector.tensor_copy(seq_f[:, t:t + 1], current[:])
                # lengths += nie
                nc.vector.tensor_add(lengths[:], lengths[:], nie[:])
                # is_end |= (current == EOS)
                eq2 = sc.tile([B, 1], f32, tag="eq2", name=f"eq2_{t}")
                nc.vector.tensor_scalar(eq2[:], current[:], float(EOS), None,
                                        ALU.is_equal)
                nc.vector.tensor_max(is_end[:], is_end[:], eq2[:])

            # ---- outputs ----
            seq_i = st.tile([B, T], i32)
            nc.vector.tensor_copy(seq_i[:], seq_f[:])
            len_i = st.tile([B, 1], i32)
            nc.vector.tensor_copy(len_i[:], lengths[:])
            nc.sync.dma_start(D_seq[:], seq_i[:])
            nc.sync.dma_start(D_len[:], len_i[:])
            nc.sync.dma_start(D_log[:], out_log[:])

    split_multiwaits(nc)
    return nc


# ---------------------------------------------------------------- runner
def make_runner(nc, n_cores):
    import jax
    from concourse.bass2jax import (_bass_exec_p, install_neuronx_cc_hook,
                                    partition_id_tensor)
    from jax.sharding import Mesh, PartitionSpec
    from jax.experimental.shard_map import shard_map

    install_neuronx_cc_hook()
    partition_name = nc.partition_id_tensor.name if nc.partition_id_tensor else None
    in_names, out_names, out_avals, zero_outs = [], [], [], []
    for alloc in nc.m.functions[0].allocations:
        if not isinstance(alloc, mybir.MemoryLocationSet):
            continue
        name = alloc.memorylocations[0].name
        if alloc.kind == "ExternalInput":
            if name != partition_name:
                in_names.append(name)
        elif alloc.kind == "ExternalOutput":
            out_names.append(name)
            shape = tuple(alloc.tensor_shape)
            dtype = mybir.dt.np(alloc.dtype)
            out_avals.append(jax.core.ShapedArray(shape, dtype))
            zero_outs.append(np.zeros(shape, dtype))
    n_params = len(in_names)
    n_outs = len(out_avals)
    all_in_names = list(in_names) + list(out_names)
    if partition_name is not None:
        all_in_names.append(partition_name)

    def _body(*args):
        operands = list(args)
        if partition_name is not None:
            operands.append(partition_id_tensor())
        outs = _bass_exec_p.bind(
            *operands,
            out_avals=tuple(out_avals),
            in_names=tuple(all_in_names),
            out_names=tuple(out_names),
            lowering_input_output_aliases=(),
            sim_require_finite=True,
            sim_require_nnan=True,
            nc=nc,
        )
        return tuple(outs)

    devices = jax.devices()[:n_cores]
    mesh = Mesh(np.asarray(devices), ("core",))
    in_specs = (PartitionSpec("core"),) * (n_params + n_outs)
    out_specs = (PartitionSpec("core"),) * n_outs
    jf = jax.jit(
        shard_map(_body, mesh=mesh, in_specs=in_specs, out_specs=out_specs,
                  check_rep=False),
        keep_unused=True,
    )

    def _concat(in_maps):
        per_core = [[np.asarray(m[n]) for n in in_names] for m in in_maps]
        concat_in = [np.concatenate([per_core[c][i] for c in range(n_cores)], axis=0)
                     for i in range(n_params)]
        concat_zero = [np.concatenate([z] * n_cores, axis=0) for z in zero_outs]
        return concat_in, concat_zero

    def run(in_maps):
        concat_in, concat_zero = _concat(in_maps)
        outs = [np.asarray(o) for o in jf(*concat_in, *concat_zero)]
        res = []
        for c in range(n_cores):
            d = {}
            for i, n in enumerate(out_names):
                per = outs[i].shape[0] // n_cores
                d[n] = outs[i][c * per:(c + 1) * per]
            res.append(d)
        return res

    def time_exec(in_maps, ncalls=8):
        """Place inputs on device once, then time pure executable dispatch+run."""
        import time as _time
        from jax.sharding import NamedSharding
        sh = NamedSharding(mesh, PartitionSpec("core"))
        concat_in, concat_zero = _concat(in_maps)
        dev_in = [jax.device_put(a, sh) for a in concat_in]
        dev_zero = [jax.device_put(a, sh) for a in concat_zero]
        ts = []
        for _ in range(ncalls):
            t0 = _time.perf_counter()
            outs = jf(*dev_in, *dev_zero)
            for o in outs:
                o.block_until_ready()
            ts.append(_time.perf_counter() - t0)
        return ts

    run.time_exec = time_exec
    return run


# ---------------------------------------------------------------- host side
def _bf16(x):
    import jax.numpy as jnp
    return np.asarray(jnp.asarray(x, jnp.bfloat16).astype(jnp.float32))


def _gumbel_table(T):
    import jax
    with jax.default_device(jax.devices("cpu")[0]):
        keys = jax.random.split(jax.random.key(1), T)
        g = jax.random.gumbel(jax.vmap(lambda k: k)(keys), (T, B_FULL, VOCAB),
                              np.float32) if False else None
        import jax.numpy as jnp
        gs = [jax.random.gumbel(keys[t], (B_FULL, VOCAB), jnp.float32) for t in range(T)]
        return np.stack([np.asarray(x) for x in gs], axis=0)  # [T, B, V]


_CACHE = {}


def _get_runner(T):
    if T not in _CACHE:
        nc = build_nc(T)
        _CACHE[T] = (nc, make_runner(nc, N_CORES))
    return _CACHE[T]


def kernel(prevs, emb, w_ih_0, w_hh_0, b_ih_0, b_hh_0,
           w_ih_1, w_hh_1, b_ih_1, b_hh_1, w_out, b_out, max_len):
    T = int(max_len)
    prevs = np.asarray(prevs, np.int32)
    emb = np.asarray(emb, np.float32)
    w_ih_0 = np.asarray(w_ih_0, np.float32); w_hh_0 = np.asarray(w_hh_0, np.float32)
    b_ih_0 = np.asarray(b_ih_0, np.float32); b_hh_0 = np.asarray(b_hh_0, np.float32)
    w_ih_1 = np.asarray(w_ih_1, np.float32); w_hh_1 = np.asarray(w_hh_1, np.float32)
    b_ih_1 = np.asarray(b_ih_1, np.float32); b_hh_1 = np.asarray(b_hh_1, np.float32)
    w_out = np.asarray(w_out, np.float32); b_out = np.asarray(b_out, np.float32)

    # host-side weight prep, packed into one blob per core
    lay, ncols = _blob_layout(T)
    base = np.zeros((128, ncols), np.float32)

    def put(name, arr, rows=128):
        a, b = lay[name]
        base[:rows, a:b] = arr

    put("ident", np.eye(128, dtype=np.float32))
    w0e = np.empty((VOCAB + 1, H4), np.float32)
    w0e[:VOCAB] = (emb.astype(np.float64) @ w_ih_0.T.astype(np.float64)).astype(np.float32)
    w0e[VOCAB] = b_ih_0 + b_hh_0
    put("w0e", w0e, rows=VOCAB + 1)

    def ktmajor(wT):  # [512, X] -> [128, 4*X] with kt-major columns
        X = wT.shape[1]
        return wT.reshape(KT, 128, X).transpose(1, 0, 2).reshape(128, KT * X)

    put("whh0", ktmajor(np.ascontiguousarray(w_hh_0.T)))
    put("wih1", ktmajor(np.ascontiguousarray(w_ih_1.T)))
    put("whh1", ktmajor(np.ascontiguousarray(w_hh_1.T)))
    put("wout", ktmajor(np.ascontiguousarray(w_out.T)))
    b1 = (b_ih_1 + b_hh_1).astype(np.float32)
    a = _bf16(b1); r = b1 - a
    bb = _bf16(r); r2 = r - bb
    cc = _bf16(r2)
    put("b13", np.stack([a, bb, cc], axis=0).astype(np.float32), rows=3)
    put("bout", np.broadcast_to(b_out, (B, VOCAB)))
    put("iota", np.arange(VOCAB, dtype=np.float32).reshape(VOCAB, 1), rows=VOCAB)

    G = _gumbel_table(T)                                          # [T, B_FULL, V]
    ga, gb = lay["gumb"]
    pa, pb = lay["prev"]
    in_maps = []
    for c in range(N_CORES):
        bs = slice(c * B, (c + 1) * B)
        blob = base.copy()
        gc = np.transpose(G[:, bs, :], (1, 0, 2))                # [B, T, V]
        blob[:, ga:gb] = (gc + b_out[None, None, :]).reshape(B, T * VOCAB)
        blob[:, pa:pb] = prevs[bs].reshape(B, 1).astype(np.float32)
        in_maps.append({"blob": blob})

    global _LAST_IN_MAPS
    _LAST_IN_MAPS = in_maps
    _, run = _get_runner(T)
    res = run(in_maps)

    seq = np.concatenate([r["seq_o"] for r in res], axis=0)       # [B, T] i32
    logits = np.concatenate([r["logits_o"] for r in res], axis=0).reshape(B_FULL, T, VOCAB)
    lengths = np.concatenate([r["len_o"] for r in res], axis=0).reshape(B_FULL)
    return seq.astype(np.int32), logits.astype(np.float32), lengths.astype(np.int32)


# revision 19
# speedup vs baseline: 70.3616x; 70.3616x over previous
"""Trainium2 Bass kernel for 2-layer LSTM token sampling (nn_ORGAN_67834713473538).

Strategy: data-parallel over the batch (1024 seqs -> 128 per core on 8 cores).
All state kept batch-major on chip; per step:
  g0 = onehot @ [emb@w_ih_0.T ; b0]  +  hA @ w_hh_0.T          (f32 matmuls)
  hA,cA = lstm_act(g0); hT_A = transpose(hA)
  g1 = b1(bf16 triple)  +  hA @ w_ih_1.T  +  hB @ w_hh_1.T
  hB,cB = lstm_act(g1); hT_B = transpose(hB)
  logits = hB @ w_out.T (+ b_out)
  sampled = argmax(logits + gumbel[t])        (gumbel precomputed on host,
                                               = jax.random.categorical exactly)
  current = where(is_end, PAD, sampled); lengths += ~is_end; is_end |= current==EOS
"""
import sys, os
for _p in ('/opt/trn_rl_repo', '/root/.axon_site/_ro/trn_rl_repo'):
    if os.path.isdir(_p) and _p not in sys.path:
        sys.path.insert(0, _p)

import numpy as np
from concourse import bass, tile, mybir

dt = mybir.dt

VOCAB, EMBED, HIDDEN, LAYERS = 30, 32, 512, 2
PAD, BOS, EOS = 0, 1, 2
B_FULL, MAX_LEN = 1024, 64
N_CORES = 8
B = B_FULL // N_CORES          # 128 per core
H4 = 4 * HIDDEN                # 2048
KT = HIDDEN // 128             # 4 k-tiles
NCH = H4 // 512                # 4 gate chunks of 512


# ---------------------------------------------------------------- wait fixup
def split_multiwaits(nc):
    """This walrus build accepts only one sync wait per instruction; hoist
    extra waits onto injected same-engine NoOps placed just before."""
    n = 0
    for f in nc.m.functions:
        for bb in f.blocks:
            out = []
            changed = False
            for ins in bb.instructions:
                si = ins.sync_info
                if si is not None and si.on_wait and len(si.on_wait) > 1:
                    ws = list(si.on_wait)
                    for w in ws[:-1]:
                        nop = mybir.InstNoOp(
                            name=f"wsplit-{nc.next_id()}",
                            engine=ins.engine,
                            sync_info=mybir.SyncInfo(on_wait=[w], on_update=[]),
                        )
                        out.append(nop)
                        n += 1
                    si.on_wait = [ws[-1]]
                    changed = True
                out.append(ins)
            if changed:
                bb.instructions = out
    return n


# ---------------------------------------------------------------- blob layout
def _blob_layout(T):
    lay, off = {}, 0
    for name, w in [("ident", 128), ("w0e", H4), ("whh0", KT * H4),
                    ("wih1", KT * H4), ("whh1", KT * H4), ("wout", KT * VOCAB),
                    ("b13", H4), ("gumb", T * VOCAB), ("bout", VOCAB),
                    ("iota", 1), ("prev", 1)]:
        lay[name] = (off, off + w)
        off += w
    return lay, off


# ---------------------------------------------------------------- device code
def build_nc(T, strip=()):
    nc = bass.Bass(target_bir_lowering=False, trn_type="TRN2")
    f32, bf16, i32, u32 = dt.float32, dt.bfloat16, dt.int32, dt.uint32
    AF = mybir.ActivationFunctionType
    ALU = mybir.AluOpType

    # single packed input blob [128, ncols]; layout mirrored in _blob_layout(T)
    lay, ncols = _blob_layout(T)
    D_blob = nc.dram_tensor("blob", [128, ncols], f32, kind="ExternalInput")

    def bl(name):
        a, b = lay[name]
        return D_blob[:, a:b]

    # single output: [logits (T*V) | seq (T) | len (1)] all f32
    D_out = nc.dram_tensor("out_o", [B, T * VOCAB + T + 1], f32,
                           kind="ExternalOutput")

    with tile.TileContext(nc) as tc:
        with tc.tile_pool(name="const", bufs=1) as cp, \
             tc.tile_pool(name="state", bufs=1) as st, \
             tc.tile_pool(name="scratch", bufs=2) as sc, \
             tc.tile_pool(name="psA", bufs=4, space="PSUM") as psA, \
             tc.tile_pool(name="psB", bufs=4, space="PSUM") as psB:

            # ---- constants / weights ----
            w0e = cp.tile([VOCAB + 1, H4], f32)
            nc.sync.dma_start(w0e[:], bl("w0e")[0:VOCAB + 1, :])
            whh0 = cp.tile([128, KT, H4], f32)
            wih1 = cp.tile([128, KT, H4], f32)
            whh1 = cp.tile([128, KT, H4], f32)
            wout = cp.tile([128, KT, VOCAB], f32)
            for kt in range(KT):
                a0, _ = lay["whh0"]
                nc.sync.dma_start(whh0[:, kt, :],
                                  D_blob[:, a0 + kt * H4:a0 + (kt + 1) * H4])
                a0, _ = lay["wih1"]
                nc.sync.dma_start(wih1[:, kt, :],
                                  D_blob[:, a0 + kt * H4:a0 + (kt + 1) * H4])
                a0, _ = lay["whh1"]
                nc.sync.dma_start(whh1[:, kt, :],
                                  D_blob[:, a0 + kt * H4:a0 + (kt + 1) * H4])
                a0, _ = lay["wout"]
                nc.sync.dma_start(wout[:, kt, :],
                                  D_blob[:, a0 + kt * VOCAB:a0 + (kt + 1) * VOCAB])
            b13f = cp.tile([3, H4], f32)
            nc.sync.dma_start(b13f[:], bl("b13")[0:3, :])
            b13 = cp.tile([3, H4], bf16)
            nc.vector.tensor_copy(b13[:], b13f[:])
            ones3 = cp.tile([3, 128], bf16)
            nc.vector.memset(ones3[:], 1.0)
            ones31 = cp.tile([1, VOCAB + 1], f32)
            nc.vector.memset(ones31[:], 1.0)
            gumb = cp.tile([B, T * VOCAB], f32)
            nc.sync.dma_start(gumb[:], bl("gumb"))
            bout = cp.tile([B, VOCAB], f32)
            nc.sync.dma_start(bout[:], bl("bout"))
            ident = cp.tile([128, 128], f32)
            nc.sync.dma_start(ident[:], bl("ident"))
            iota30 = cp.tile([VOCAB, 1], f32)
            nc.sync.dma_start(iota30[:], bl("iota")[0:VOCAB, :])

            # ---- state ----
            cA = st.tile([B, HIDDEN], f32)
            cB = st.tile([B, HIDDEN], f32)
            hTA = st.tile([128, KT, 128], f32)
            hTB = st.tile([128, KT, 128], f32)
            nc.vector.memset(cA[:], 0.0)
            nc.vector.memset(cB[:], 0.0)
            nc.vector.memset(hTA[:], 0.0)
            nc.vector.memset(hTB[:], 0.0)
            current = st.tile([B, 1], f32)
            is_end = st.tile([B, 1], f32)
            lengths = st.tile([B, 1], f32)
            nc.vector.memset(lengths[:], 0.0)
            nc.sync.dma_start(current[:], bl("prev"))
            nc.vector.tensor_scalar(is_end[:], current[:], float(EOS), None,
                                    ALU.is_equal)
            onehot = st.tile([VOCAB + 1, 128], f32)
            nc.vector.memset(onehot[:], 1.0)   # row 30 stays 1.0 forever
            seq_f = st.tile([B, T], f32)
            out_log = st.tile([B, T * VOCAB], f32)
            if strip:
                nc.vector.memset(seq_f[:], 0.0)
                nc.vector.memset(out_log[:], 0.0)

            for t in range(T):
                # ---- onehot(current) ----
                if "onehot" not in strip:
                    curT = psB.tile([1, 128], f32, tag="psB", name=f"curT{t}")
                    nc.tensor.transpose(curT[:], current[:], ident[:])
                    cur_row = sc.tile([1, 128], f32, tag="cur_row", name=f"cur_row{t}")
                    nc.vector.tensor_copy(cur_row[:], curT[:])
                    bc = psB.tile([VOCAB + 1, 128], f32, tag="psB", name=f"bc{t}")
                    nc.tensor.matmul(bc[:], ones31[:], cur_row[:], start=True, stop=True)
                    nc.vector.tensor_scalar(onehot[0:VOCAB, :], bc[0:VOCAB, :],
                                            iota30[:], None, ALU.is_equal)

                # ---- layer 0 gates ----
                g0c = []
                for c in range(NCH):
                    g = psA.tile([B, 512], f32, tag="psA", name=f"g0_{t}_{c}")
                    g0c.append(g)
                    cs = slice(c * 512, (c + 1) * 512)
                    if "pp" not in strip:
                        nc.tensor.matmul(g[:], onehot[:], w0e[:, cs], start=True,
                                         stop=False)
                    for kt in range(KT):
                        nc.tensor.matmul(g[:], hTA[:, kt, :], whh0[:, kt, cs],
                                         start=("pp" in strip and kt == 0),
                                         stop=(kt == KT - 1))

                # ---- layer 0 activations ----
                sig_i = sc.tile([B, 512], f32, tag="sig_i", name=f"sig_i{t}")
                sig_f = sc.tile([B, 512], f32, tag="sig_f", name=f"sig_f{t}")
                tnh_g = sc.tile([B, 512], f32, tag="tnh_g", name=f"tnh_g{t}")
                sig_o = sc.tile([B, 512], f32, tag="sig_o", name=f"sig_o{t}")
                nc.scalar.activation(sig_i[:], g0c[0][:], AF.Sigmoid)
                nc.scalar.activation(sig_f[:], g0c[1][:], AF.Sigmoid)
                nc.scalar.activation(tnh_g[:], g0c[2][:], AF.Tanh)
                nc.scalar.activation(sig_o[:], g0c[3][:], AF.Sigmoid)
                t1 = sc.tile([B, 512], f32, tag="t1", name=f"t1_{t}")
                t2 = sc.tile([B, 512], f32, tag="t2", name=f"t2_{t}")
                nc.vector.tensor_mul(t1[:], sig_i[:], tnh_g[:])
                nc.vector.tensor_mul(t2[:], sig_f[:], cA[:])
                nc.vector.tensor_add(cA[:], t1[:], t2[:])
                tnh_c = sc.tile([B, 512], f32, tag="tnh_c", name=f"tnh_c{t}")
                nc.scalar.activation(tnh_c[:], cA[:], AF.Tanh)
                hA = sc.tile([B, 512], f32, tag="hA", name=f"hA{t}")
                nc.vector.tensor_mul(hA[:], sig_o[:], tnh_c[:])

                # ---- transpose hA -> hTA ----
                for kt in range(KT):
                    tp = psA.tile([128, 128], f32, tag="psA", name=f"tpa{t}_{kt}")
                    nc.tensor.transpose(tp[:], hA[:, kt * 128:(kt + 1) * 128], ident[:])
                    nc.vector.tensor_copy(hTA[:, kt, :], tp[:])

                # ---- layer 1 gates ----
                g1c = []
                for c in range(NCH):
                    g = psB.tile([B, 512], f32, tag="psB", name=f"g1_{t}_{c}")
                    g1c.append(g)
                    cs = slice(c * 512, (c + 1) * 512)
                    if "pp" not in strip:
                        nc.tensor.matmul(g[:], ones3[:], b13[:, cs], start=True,
                                         stop=False)
                    for kt in range(KT):
                        nc.tensor.matmul(g[:], hTA[:, kt, :], wih1[:, kt, cs],
                                         start=("pp" in strip and kt == 0),
                                         stop=False)
                    for kt in range(KT):
                        nc.tensor.matmul(g[:], hTB[:, kt, :], whh1[:, kt, cs],
                                         start=False, stop=(kt == KT - 1))

                # ---- layer 1 activations ----
                sig_i1 = sc.tile([B, 512], f32, tag="sig_i", name=f"sig_i1_{t}")
                sig_f1 = sc.tile([B, 512], f32, tag="sig_f", name=f"sig_f1_{t}")
                tnh_g1 = sc.tile([B, 512], f32, tag="tnh_g", name=f"tnh_g1_{t}")
                sig_o1 = sc.tile([B, 512], f32, tag="sig_o", name=f"sig_o1_{t}")
                nc.scalar.activation(sig_i1[:], g1c[0][:], AF.Sigmoid)
                nc.scalar.activation(sig_f1[:], g1c[1][:], AF.Sigmoid)
                nc.scalar.activation(tnh_g1[:], g1c[2][:], AF.Tanh)
                nc.scalar.activation(sig_o1[:], g1c[3][:], AF.Sigmoid)
                t11 = sc.tile([B, 512], f32, tag="t1", name=f"t11_{t}")
                t21 = sc.tile([B, 512], f32, tag="t2", name=f"t21_{t}")
                nc.vector.tensor_mul(t11[:], sig_i1[:], tnh_g1[:])
                nc.vector.tensor_mul(t21[:], sig_f1[:], cB[:])
                nc.vector.tensor_add(cB[:], t11[:], t21[:])
                tnh_c1 = sc.tile([B, 512], f32, tag="tnh_c", name=f"tnh_c1_{t}")
                nc.scalar.activation(tnh_c1[:], cB[:], AF.Tanh)
                hB = sc.tile([B, 512], f32, tag="hB", name=f"hB{t}")
                nc.vector.tensor_mul(hB[:], sig_o1[:], tnh_c1[:])

                # ---- transpose hB -> hTB ----
                for kt in range(KT):
                    tp = psB.tile([128, 128], f32, tag="psB", name=f"tpb{t}_{kt}")
                    nc.tensor.transpose(tp[:], hB[:, kt * 128:(kt + 1) * 128], ident[:])
                    nc.vector.tensor_copy(hTB[:, kt, :], tp[:])

                # ---- logits ----
                if "logits" not in strip:
                    L = psB.tile([B, VOCAB], f32, tag="psB", name=f"L{t}")
                    for kt in range(KT):
                        nc.tensor.matmul(L[:], hTB[:, kt, :], wout[:, kt, :],
                                         start=(kt == 0), stop=(kt == KT - 1))
                    ts30 = slice(t * VOCAB, (t + 1) * VOCAB)
                    nc.vector.tensor_add(out_log[:, ts30], L[:], bout[:])
                if "samp" not in strip:
                    scores = sc.tile([B, VOCAB], f32, tag="scores", name=f"scores{t}")
                    nc.vector.tensor_add(scores[:], L[:], gumb[:, ts30])

                    # ---- sample: argmax(scores) ----
                    mx8 = sc.tile([B, 8], f32, tag="mx8", name=f"mx8_{t}")
                    idx8 = sc.tile([B, 8], u32, tag="idx8", name=f"idx8_{t}")
                    nc.vector.max(mx8[:], scores[:])
                    nc.vector.max_index(idx8[:], mx8[:], scores[:])
                    samp = sc.tile([B, 1], f32, tag="samp", name=f"samp{t}")
                    nc.vector.tensor_copy(samp[:], idx8[:, 0:1])

                    # nie = (is_end == 0);  current = samp * nie  (PAD == 0)
                    nie = sc.tile([B, 1], f32, tag="nie", name=f"nie{t}")
                    nc.vector.tensor_scalar(nie[:], is_end[:], 0.0, None, ALU.is_equal)
                    nc.vector.tensor_mul(current[:], samp[:], nie[:])
                    nc.vector.tensor_copy(seq_f[:, t:t + 1], current[:])
                    # lengths += nie
                    nc.vector.tensor_add(lengths[:], lengths[:], nie[:])
                    # is_end |= (current == EOS)
                    eq2 = sc.tile([B, 1], f32, tag="eq2", name=f"eq2_{t}")
                    nc.vector.tensor_scalar(eq2[:], current[:], float(EOS), None,
                                            ALU.is_equal)
                    nc.vector.tensor_max(is_end[:], is_end[:], eq2[:])

            # ---- outputs (packed into one tensor) ----
            nc.sync.dma_start(D_out[:, 0:T * VOCAB], out_log[:])
            nc.sync.dma_start(D_out[:, T * VOCAB:T * VOCAB + T], seq_f[:])
            nc.sync.dma_start(D_out[:, T * VOCAB + T:T * VOCAB + T + 1], lengths[:])

    split_multiwaits(nc)
    return nc


# ---------------------------------------------------------------- runner
def make_runner(nc, n_cores):
    import jax
    from concourse.bass2jax import (_bass_exec_p, install_neuronx_cc_hook,
                                    partition_id_tensor)
    from jax.sharding import Mesh, PartitionSpec
    from jax.experimental.shard_map import shard_map

    install_neuronx_cc_hook()
    partition_name = nc.partition_id_tensor.name if nc.partition_id_tensor else None
    in_names, out_names, out_avals, zero_outs = [], [], [], []
    for alloc in nc.m.functions[0].allocations:
        if not isinstance(alloc, mybir.MemoryLocationSet):
            continue
        name = alloc.memorylocations[0].name
        if alloc.kind == "ExternalInput":
            if name != partition_name:
                in_names.append(name)
        elif alloc.kind == "ExternalOutput":
            out_names.append(name)
            shape = tuple(alloc.tensor_shape)
            dtype = mybir.dt.np(alloc.dtype)
            out_avals.append(jax.core.ShapedArray(shape, dtype))
            zero_outs.append(np.zeros(shape, dtype))
    n_params = len(in_names)
    n_outs = len(out_avals)
    all_in_names = list(in_names) + list(out_names)
    if partition_name is not None:
        all_in_names.append(partition_name)

    def _body(*args):
        operands = list(args)
        if partition_name is not None:
            operands.append(partition_id_tensor())
        outs = _bass_exec_p.bind(
            *operands,
            out_avals=tuple(out_avals),
            in_names=tuple(all_in_names),
            out_names=tuple(out_names),
            lowering_input_output_aliases=(),
            sim_require_finite=True,
            sim_require_nnan=True,
            nc=nc,
        )
        return tuple(outs)

    devices = jax.devices()[:n_cores]
    mesh = Mesh(np.asarray(devices), ("core",))
    in_specs = (PartitionSpec("core"),) * (n_params + n_outs)
    out_specs = (PartitionSpec("core"),) * n_outs
    jf = jax.jit(
        shard_map(_body, mesh=mesh, in_specs=in_specs, out_specs=out_specs,
                  check_rep=False),
        keep_unused=True,
    )

    def _concat(in_maps):
        per_core = [[np.asarray(m[n]) for n in in_names] for m in in_maps]
        concat_in = [np.concatenate([per_core[c][i] for c in range(n_cores)], axis=0)
                     for i in range(n_params)]
        concat_zero = [np.concatenate([z] * n_cores, axis=0) for z in zero_outs]
        return concat_in, concat_zero

    def run(in_maps):
        concat_in, concat_zero = _concat(in_maps)
        outs = [np.asarray(o) for o in jf(*concat_in, *concat_zero)]
        res = []
        for c in range(n_cores):
            d = {}
            for i, n in enumerate(out_names):
                per = outs[i].shape[0] // n_cores
                d[n] = outs[i][c * per:(c + 1) * per]
            res.append(d)
        return res

    def time_exec(in_maps, ncalls=8):
        """Place inputs on device once, then time pure executable dispatch+run."""
        import time as _time
        from jax.sharding import NamedSharding
        sh = NamedSharding(mesh, PartitionSpec("core"))
        concat_in, concat_zero = _concat(in_maps)
        dev_in = [jax.device_put(a, sh) for a in concat_in]
        dev_zero = [jax.device_put(a, sh) for a in concat_zero]
        ts = []
        for _ in range(ncalls):
            t0 = _time.perf_counter()
            outs = jf(*dev_in, *dev_zero)
            for o in outs:
                o.block_until_ready()
            ts.append(_time.perf_counter() - t0)
        return ts

    run.time_exec = time_exec
    return run


# ---------------------------------------------------------------- host side
def _bf16(x):
    import jax.numpy as jnp
    return np.asarray(jnp.asarray(x, jnp.bfloat16).astype(jnp.float32))


def _gumbel_table(T):
    import jax
    with jax.default_device(jax.devices("cpu")[0]):
        keys = jax.random.split(jax.random.key(1), T)
        g = jax.random.gumbel(jax.vmap(lambda k: k)(keys), (T, B_FULL, VOCAB),
                              np.float32) if False else None
        import jax.numpy as jnp
        gs = [jax.random.gumbel(keys[t], (B_FULL, VOCAB), jnp.float32) for t in range(T)]
        return np.stack([np.asarray(x) for x in gs], axis=0)  # [T, B, V]


_CACHE = {}


def _get_runner(T):
    if T not in _CACHE:
        nc = build_nc(T)
        _CACHE[T] = (nc, make_runner(nc, N_CORES))
    return _CACHE[T]


def kernel(prevs, emb, w_ih_0, w_hh_0, b_ih_0, b_hh_0,
           w_ih_1, w_hh_1, b_ih_1, b_hh_1, w_out, b_out, max_len):
    T = int(max_len)
    prevs = np.asarray(prevs, np.int32)
    emb = np.asarray(emb, np.float32)
    w_ih_0 = np.asarray(w_ih_0, np.float32); w_hh_0 = np.asarray(w_hh_0, np.float32)
    b_ih_0 = np.asarray(b_ih_0, np.float32); b_hh_0 = np.asarray(b_hh_0, np.float32)
    w_ih_1 = np.asarray(w_ih_1, np.float32); w_hh_1 = np.asarray(w_hh_1, np.float32)
    b_ih_1 = np.asarray(b_ih_1, np.float32); b_hh_1 = np.asarray(b_hh_1, np.float32)
    w_out = np.asarray(w_out, np.float32); b_out = np.asarray(b_out, np.float32)

    # host-side weight prep, packed into one blob per core
    lay, ncols = _blob_layout(T)
    base = np.zeros((128, ncols), np.float32)

    def put(name, arr, rows=128):
        a, b = lay[name]
        base[:rows, a:b] = arr

    put("ident", np.eye(128, dtype=np.float32))
    w0e = np.empty((VOCAB + 1, H4), np.float32)
    w0e[:VOCAB] = (emb.astype(np.float64) @ w_ih_0.T.astype(np.float64)).astype(np.float32)
    w0e[VOCAB] = b_ih_0 + b_hh_0
    put("w0e", w0e, rows=VOCAB + 1)

    def ktmajor(wT):  # [512, X] -> [128, 4*X] with kt-major columns
        X = wT.shape[1]
        return wT.reshape(KT, 128, X).transpose(1, 0, 2).reshape(128, KT * X)

    put("whh0", ktmajor(np.ascontiguousarray(w_hh_0.T)))
    put("wih1", ktmajor(np.ascontiguousarray(w_ih_1.T)))
    put("whh1", ktmajor(np.ascontiguousarray(w_hh_1.T)))
    put("wout", ktmajor(np.ascontiguousarray(w_out.T)))
    b1 = (b_ih_1 + b_hh_1).astype(np.float32)
    a = _bf16(b1); r = b1 - a
    bb = _bf16(r); r2 = r - bb
    cc = _bf16(r2)
    put("b13", np.stack([a, bb, cc], axis=0).astype(np.float32), rows=3)
    put("bout", np.broadcast_to(b_out, (B, VOCAB)))
    put("iota", np.arange(VOCAB, dtype=np.float32).reshape(VOCAB, 1), rows=VOCAB)

    G = _gumbel_table(T)                                          # [T, B_FULL, V]
    ga, gb = lay["gumb"]
    pa, pb = lay["prev"]
    in_maps = []
    for c in range(N_CORES):
        bs = slice(c * B, (c + 1) * B)
        blob = base.copy()
        gc = np.transpose(G[:, bs, :], (1, 0, 2))                # [B, T, V]
        blob[:, ga:gb] = (gc + b_out[None, None, :]).reshape(B, T * VOCAB)
        blob[:, pa:pb] = prevs[bs].reshape(B, 1).astype(np.float32)
        in_maps.append({"blob": blob})

    global _LAST_IN_MAPS
    _LAST_IN_MAPS = in_maps
    _, run = _get_runner(T)
    res = run(in_maps)

    out = np.concatenate([r["out_o"] for r in res], axis=0)       # [B, T*V+T+1] f32
    logits = out[:, :T * VOCAB].reshape(B_FULL, T, VOCAB).astype(np.float32)
    seq = np.rint(out[:, T * VOCAB:T * VOCAB + T]).astype(np.int32)
    lengths = np.rint(out[:, T * VOCAB + T]).astype(np.int32)
    return seq, logits, lengths


# revision 37
# speedup vs baseline: 119.6006x; 1.6998x over previous
"""Trainium2 Bass kernel for 2-layer LSTM token sampling (nn_ORGAN_67834713473538).

Strategy: data-parallel over the batch (1024 seqs -> 128 per core on 8 cores).
All state kept batch-major on chip; per step:
  g0 = onehot @ [emb@w_ih_0.T ; b0]  +  hA @ w_hh_0.T          (f32 matmuls)
  hA,cA = lstm_act(g0); hT_A = transpose(hA)
  g1 = b1(bf16 triple)  +  hA @ w_ih_1.T  +  hB @ w_hh_1.T
  hB,cB = lstm_act(g1); hT_B = transpose(hB)
  logits = hB @ w_out.T (+ b_out)
  sampled = argmax(logits + gumbel[t])        (gumbel precomputed on host,
                                               = jax.random.categorical exactly)
  current = where(is_end, PAD, sampled); lengths += ~is_end; is_end |= current==EOS
"""
import sys, os
for _p in ('/opt/trn_rl_repo', '/root/.axon_site/_ro/trn_rl_repo'):
    if os.path.isdir(_p) and _p not in sys.path:
        sys.path.insert(0, _p)

import numpy as np
from concourse import bass, tile, mybir

dt = mybir.dt

VOCAB, EMBED, HIDDEN, LAYERS = 30, 32, 512, 2
PAD, BOS, EOS = 0, 1, 2
B_FULL, MAX_LEN = 1024, 64
N_CORES = 8
B = B_FULL // N_CORES          # 128 per core
H4 = 4 * HIDDEN                # 2048
KT = HIDDEN // 128             # 4 k-tiles
NCH = H4 // 512                # 4 gate chunks of 512


# ---------------------------------------------------------------- wait fixup
def split_multiwaits(nc):
    """This walrus build accepts only one sync wait per instruction; hoist
    extra waits onto injected same-engine NoOps placed just before."""
    n = 0
    for f in nc.m.functions:
        for bb in f.blocks:
            out = []
            changed = False
            for ins in bb.instructions:
                si = ins.sync_info
                if si is not None and si.on_wait and len(si.on_wait) > 1:
                    ws = list(si.on_wait)
                    for w in ws[:-1]:
                        nop = mybir.InstNoOp(
                            name=f"wsplit-{nc.next_id()}",
                            engine=ins.engine,
                            sync_info=mybir.SyncInfo(on_wait=[w], on_update=[]),
                        )
                        out.append(nop)
                        n += 1
                    si.on_wait = [ws[-1]]
                    changed = True
                out.append(ins)
            if changed:
                bb.instructions = out
    return n


# ---------------------------------------------------------------- blob layout
def _blob_layout(T):
    lay, off = {}, 0
    for name, w in [("ident", 128), ("w0e", H4), ("whh0", KT * H4),
                    ("wih1", KT * H4), ("whh1", KT * H4), ("wout", KT * VOCAB),
                    ("b13", H4), ("gumb", T * VOCAB), ("bout", VOCAB),
                    ("iota", 1), ("prev", 1)]:
        lay[name] = (off, off + w)
        off += w
    return lay, off


# ---------------------------------------------------------------- device code
def build_nc(T, strip=(), hybrid=True):
    nc = bass.Bass(target_bir_lowering=False, trn_type="TRN2")
    f32, bf16, i32, u32 = dt.float32, dt.bfloat16, dt.int32, dt.uint32
    AF = mybir.ActivationFunctionType
    ALU = mybir.AluOpType

    # single packed input blob [128, ncols]; layout mirrored in _blob_layout(T)
    lay, ncols = _blob_layout(T)
    D_blob = nc.dram_tensor("blob", [128, ncols], f32, kind="ExternalInput")

    def bl(name):
        a, b = lay[name]
        return D_blob[:, a:b]

    # single output: [logits (T*V) | seq (T) | len (1)] all f32
    D_out = nc.dram_tensor("out_o", [B, T * VOCAB + T + 1], f32,
                           kind="ExternalOutput")

    with tile.TileContext(nc) as tc:
        with tc.tile_pool(name="const", bufs=1) as cp, \
             tc.tile_pool(name="state", bufs=1) as st, \
             tc.tile_pool(name="scratch", bufs=2) as sc, \
             tc.tile_pool(name="psA", bufs=4, space="PSUM") as psA, \
             tc.tile_pool(name="psB", bufs=4, space="PSUM") as psB:

            # ---- constants / weights ----
            f32r = dt.float32r
            if hybrid:
                # split each big weight matrix into f32r-hi + bf16-lo on device
                whi = {}
                wlo = {}
                with tc.tile_pool(name="cvt", bufs=1) as cvt:
                    for nm in ("whh0", "wih1", "whh1"):
                        whi[nm] = cp.tile([128, KT, H4], f32r, name=f"{nm}_hi")
                        wlo[nm] = cp.tile([128, KT, H4], bf16, name=f"{nm}_lo")
                        a0, _ = lay[nm]
                        for kt in range(KT):
                            tmp = cvt.tile([128, H4], f32, tag="cvt",
                                           name=f"cvt_{nm}_{kt}")
                            nc.sync.dma_start(
                                tmp[:], D_blob[:, a0 + kt * H4:a0 + (kt + 1) * H4])
                            nc.vector.tensor_copy(whi[nm][:, kt, :], tmp[:])
                            nc.vector.tensor_sub(
                                wlo[nm][:, kt, :], tmp[:],
                                whi[nm][:, kt, :].bitcast(f32))
                    w0e_hi = cp.tile([VOCAB + 1, H4], f32r)
                    w0e_lo = cp.tile([VOCAB + 1, H4], bf16)
                    tmp0 = cvt.tile([VOCAB + 1, H4], f32, tag="cvt", name="cvt_w0e")
                    nc.sync.dma_start(tmp0[:], bl("w0e")[0:VOCAB + 1, :])
                    nc.vector.tensor_copy(w0e_hi[:], tmp0[:])
                    nc.vector.tensor_sub(w0e_lo[:], tmp0[:], w0e_hi[:].bitcast(f32))
                whh0 = wih1 = whh1 = w0e = None
            else:
                w0e = cp.tile([VOCAB + 1, H4], f32)
                nc.sync.dma_start(w0e[:], bl("w0e")[0:VOCAB + 1, :])
                whh0 = cp.tile([128, KT, H4], f32)
                wih1 = cp.tile([128, KT, H4], f32)
                whh1 = cp.tile([128, KT, H4], f32)
                for kt in range(KT):
                    for nm, tl in (("whh0", whh0), ("wih1", wih1), ("whh1", whh1)):
                        a0, _ = lay[nm]
                        nc.sync.dma_start(
                            tl[:, kt, :], D_blob[:, a0 + kt * H4:a0 + (kt + 1) * H4])
            wout = cp.tile([128, KT, VOCAB], f32)
            for kt in range(KT):
                a0, _ = lay["wout"]
                nc.sync.dma_start(wout[:, kt, :],
                                  D_blob[:, a0 + kt * VOCAB:a0 + (kt + 1) * VOCAB])
            b13 = cp.tile([3, H4], bf16)
            with tc.tile_pool(name="cvtb", bufs=1) as cvtb:
                b13f = cvtb.tile([3, H4], f32)
                nc.sync.dma_start(b13f[:], bl("b13")[0:3, :])
                nc.vector.tensor_copy(b13[:], b13f[:])
            ones3 = cp.tile([3, 128], bf16)
            nc.vector.memset(ones3[:], 1.0)
            ones31 = cp.tile([1, VOCAB + 1], f32)
            nc.vector.memset(ones31[:], 1.0)
            if hybrid:
                gumb = None     # streamed per step
            else:
                gumb = cp.tile([B, T * VOCAB], f32)
                nc.sync.dma_start(gumb[:], bl("gumb"))
            bout = cp.tile([B, VOCAB], f32)
            nc.sync.dma_start(bout[:], bl("bout"))
            ident = cp.tile([128, 128], f32)
            nc.sync.dma_start(ident[:], bl("ident"))
            iota30 = cp.tile([VOCAB, 1], f32)
            nc.sync.dma_start(iota30[:], bl("iota")[0:VOCAB, :])

            # ---- state ----
            cA = st.tile([B, HIDDEN], f32)
            cB = st.tile([B, HIDDEN], f32)
            nc.vector.memset(cA[:], 0.0)
            nc.vector.memset(cB[:], 0.0)
            if hybrid:
                hTA_hi = st.tile([128, KT, 128], dt.float32r)
                hTA_lo = st.tile([128, KT, 128], dt.float32r)
                hTA_b = st.tile([128, KT, 128], bf16)
                hTB_hi = st.tile([128, KT, 128], dt.float32r)
                hTB_lo = st.tile([128, KT, 128], dt.float32r)
                hTB_b = st.tile([128, KT, 128], bf16)
                hTB = st.tile([128, KT, 128], f32)   # f32 copy for logits
                for tl in (hTA_b, hTB_b, hTB):
                    nc.vector.memset(tl[:], 0.0)
                # memset cannot write f32r; zero via (cA * 0) copy-convert
                for tl in (hTA_hi, hTA_lo, hTB_hi, hTB_lo):
                    nc.vector.tensor_scalar(tl[:], cA[:, 0:KT * 128], 0.0, None,
                                            ALU.mult)
            else:
                hTA = st.tile([128, KT, 128], f32)
                hTB = st.tile([128, KT, 128], f32)
                nc.vector.memset(hTA[:], 0.0)
                nc.vector.memset(hTB[:], 0.0)
            current = st.tile([B, 1], f32)
            is_end = st.tile([B, 1], f32)
            lengths = st.tile([B, 1], f32)
            nc.vector.memset(lengths[:], 0.0)
            nc.sync.dma_start(current[:], bl("prev"))
            nc.vector.tensor_scalar(is_end[:], current[:], float(EOS), None,
                                    ALU.is_equal)
            if hybrid:
                onehot = st.tile([VOCAB + 1, 128], dt.float32r)
                onehot_b = st.tile([VOCAB + 1, 128], bf16)
                nc.vector.memset(onehot_b[:], 1.0)
                # all-ones init (row 30 stays 1.0 forever); f32r via ts convert
                nc.vector.tensor_scalar(onehot[:], ident[0:VOCAB + 1, :], 0.0, 1.0,
                                        ALU.mult, ALU.add)
            else:
                onehot = st.tile([VOCAB + 1, 128], f32)
                onehot_b = None
                nc.vector.memset(onehot[:], 1.0)   # row 30 stays 1.0 forever
            seq_f = st.tile([B, T], f32)
            out_log = None if hybrid else st.tile([B, T * VOCAB], f32)
            if strip:
                nc.vector.memset(seq_f[:], 0.0)

            for t in range(T):
                # ---- onehot(current) ----
                if "onehot" not in strip:
                    curT = psB.tile([1, 128], f32, tag="psB", name=f"curT{t}")
                    nc.tensor.transpose(curT[:], current[:], ident[:])
                    cur_row = sc.tile([1, 128], f32, tag="cur_row", name=f"cur_row{t}")
                    nc.vector.tensor_copy(cur_row[:], curT[:])
                    bc = psB.tile([VOCAB + 1, 128], f32, tag="psB", name=f"bc{t}")
                    nc.tensor.matmul(bc[:], ones31[:], cur_row[:], start=True, stop=True)
                    nc.vector.tensor_scalar(onehot[0:VOCAB, :], bc[0:VOCAB, :],
                                            iota30[:], None, ALU.is_equal)
                    if hybrid:
                        nc.vector.tensor_copy(onehot_b[0:VOCAB, :],
                                              onehot[0:VOCAB, :].bitcast(f32))

                # ---- layer 0 gates ----
                g0c = []
                for c in range(NCH):
                    g = psA.tile([B, 512], f32, tag="psA", name=f"g0_{t}_{c}")
                    g0c.append(g)
                    cs = slice(c * 512, (c + 1) * 512)
                    if hybrid:
                        nc.tensor.matmul(g[:], onehot[:], w0e_hi[:, cs],
                                         start=True, stop=False)
                        nc.tensor.matmul(g[:], onehot_b[:], w0e_lo[:, cs],
                                         start=False, stop=False)
                        for kt in range(KT):
                            nc.tensor.matmul(g[:], hTA_hi[:, kt, :],
                                             whi["whh0"][:, kt, cs],
                                             start=False, stop=False)
                        for kt in range(KT):
                            nc.tensor.matmul(g[:], hTA_lo[:, kt, :],
                                             whi["whh0"][:, kt, cs],
                                             start=False, stop=False)
                        for kt in range(KT):
                            nc.tensor.matmul(g[:], hTA_b[:, kt, :],
                                             wlo["whh0"][:, kt, cs],
                                             start=False, stop=(kt == KT - 1))
                    else:
                        if "pp" not in strip:
                            nc.tensor.matmul(g[:], onehot[:], w0e[:, cs], start=True,
                                             stop=False)
                        for kt in range(KT):
                            nc.tensor.matmul(g[:], hTA[:, kt, :], whh0[:, kt, cs],
                                             start=("pp" in strip and kt == 0),
                                             stop=(kt == KT - 1))

                # ---- layer 0 activations ----
                sig_i = sc.tile([B, 512], f32, tag="sig_i", name=f"sig_i{t}")
                sig_f = sc.tile([B, 512], f32, tag="sig_f", name=f"sig_f{t}")
                tnh_g = sc.tile([B, 512], f32, tag="tnh_g", name=f"tnh_g{t}")
                sig_o = sc.tile([B, 512], f32, tag="sig_o", name=f"sig_o{t}")
                nc.scalar.activation(sig_i[:], g0c[0][:], AF.Sigmoid)
                nc.scalar.activation(sig_f[:], g0c[1][:], AF.Sigmoid)
                nc.scalar.activation(tnh_g[:], g0c[2][:], AF.Tanh)
                nc.scalar.activation(sig_o[:], g0c[3][:], AF.Sigmoid)
                nc.vector.tensor_mul(sig_i[:], sig_i[:], tnh_g[:])   # i*tanh(g)
                nc.vector.tensor_mul(sig_f[:], sig_f[:], cA[:])      # f*c
                nc.vector.tensor_add(cA[:], sig_i[:], sig_f[:])      # c_new
                nc.scalar.activation(tnh_g[:], cA[:], AF.Tanh)       # tanh(c_new)
                hA = sig_o
                nc.vector.tensor_mul(hA[:], sig_o[:], tnh_g[:])      # h = o*tanh(c)

                # ---- transpose hA -> hTA (+ precision splits) ----
                for kt in range(KT):
                    tp = psA.tile([128, 128], f32, tag="psA", name=f"tpa{t}_{kt}")
                    nc.tensor.transpose(tp[:], hA[:, kt * 128:(kt + 1) * 128], ident[:])
                    if hybrid:
                        nc.vector.tensor_copy(hTA_hi[:, kt, :], tp[:])
                        nc.vector.tensor_sub(hTA_lo[:, kt, :], tp[:],
                                             hTA_hi[:, kt, :].bitcast(f32))
                        nc.vector.tensor_copy(hTA_b[:, kt, :], tp[:])
                    else:
                        nc.vector.tensor_copy(hTA[:, kt, :], tp[:])

                # ---- layer 1 gates ----
                g1c = []
                for c in range(NCH):
                    g = psB.tile([B, 512], f32, tag="psB", name=f"g1_{t}_{c}")
                    g1c.append(g)
                    cs = slice(c * 512, (c + 1) * 512)
                    nc.tensor.matmul(g[:], ones3[:], b13[:, cs], start=True,
                                     stop=False)
                    if hybrid:
                        for hi_t, lo_t, b_t, nm in ((hTA_hi, hTA_lo, hTA_b, "wih1"),
                                                    (hTB_hi, hTB_lo, hTB_b, "whh1")):
                            for kt in range(KT):
                                nc.tensor.matmul(g[:], hi_t[:, kt, :],
                                                 whi[nm][:, kt, cs],
                                                 start=False, stop=False)
                            for kt in range(KT):
                                nc.tensor.matmul(g[:], lo_t[:, kt, :],
                                                 whi[nm][:, kt, cs],
                                                 start=False, stop=False)
                            for kt in range(KT):
                                nc.tensor.matmul(g[:], b_t[:, kt, :],
                                                 wlo[nm][:, kt, cs],
                                                 start=False,
                                                 stop=(nm == "whh1" and kt == KT - 1))
                    else:
                        for kt in range(KT):
                            nc.tensor.matmul(g[:], hTA[:, kt, :], wih1[:, kt, cs],
                                             start=False, stop=False)
                        for kt in range(KT):
                            nc.tensor.matmul(g[:], hTB[:, kt, :], whh1[:, kt, cs],
                                             start=False, stop=(kt == KT - 1))

                # ---- layer 1 activations ----
                sig_i1 = sc.tile([B, 512], f32, tag="sig_i", name=f"sig_i1_{t}")
                sig_f1 = sc.tile([B, 512], f32, tag="sig_f", name=f"sig_f1_{t}")
                tnh_g1 = sc.tile([B, 512], f32, tag="tnh_g", name=f"tnh_g1_{t}")
                sig_o1 = sc.tile([B, 512], f32, tag="sig_o", name=f"sig_o1_{t}")
                nc.scalar.activation(sig_i1[:], g1c[0][:], AF.Sigmoid)
                nc.scalar.activation(sig_f1[:], g1c[1][:], AF.Sigmoid)
                nc.scalar.activation(tnh_g1[:], g1c[2][:], AF.Tanh)
                nc.scalar.activation(sig_o1[:], g1c[3][:], AF.Sigmoid)
                nc.vector.tensor_mul(sig_i1[:], sig_i1[:], tnh_g1[:])
                nc.vector.tensor_mul(sig_f1[:], sig_f1[:], cB[:])
                nc.vector.tensor_add(cB[:], sig_i1[:], sig_f1[:])
                nc.scalar.activation(tnh_g1[:], cB[:], AF.Tanh)
                hB = sig_o1
                nc.vector.tensor_mul(hB[:], sig_o1[:], tnh_g1[:])

                # ---- transpose hB -> hTB (+ precision splits) ----
                for kt in range(KT):
                    tp = psB.tile([128, 128], f32, tag="psB", name=f"tpb{t}_{kt}")
                    nc.tensor.transpose(tp[:], hB[:, kt * 128:(kt + 1) * 128], ident[:])
                    nc.vector.tensor_copy(hTB[:, kt, :], tp[:])
                    if hybrid:
                        nc.vector.tensor_copy(hTB_hi[:, kt, :], tp[:])
                        nc.vector.tensor_sub(hTB_lo[:, kt, :], tp[:],
                                             hTB_hi[:, kt, :].bitcast(f32))
                        nc.vector.tensor_copy(hTB_b[:, kt, :], tp[:])

                # ---- logits ----
                if "logits" not in strip:
                    L = psB.tile([B, VOCAB], f32, tag="psB", name=f"L{t}")
                    for kt in range(KT):
                        nc.tensor.matmul(L[:], hTB[:, kt, :], wout[:, kt, :],
                                         start=(kt == 0), stop=(kt == KT - 1))
                    ts30 = slice(t * VOCAB, (t + 1) * VOCAB)
                    if hybrid:
                        olog = sc.tile([B, VOCAB], f32, tag="olog", name=f"olog{t}")
                        nc.vector.tensor_add(olog[:], L[:], bout[:])
                        nc.sync.dma_start(D_out[:, ts30], olog[:])
                    else:
                        nc.vector.tensor_add(out_log[:, ts30], L[:], bout[:])
                if "samp" not in strip:
                    if hybrid:
                        ga0, _ = lay["gumb"]
                        gt = sc.tile([B, VOCAB], f32, tag="gt", name=f"gt{t}")
                        nc.sync.dma_start(
                            gt[:], D_blob[:, ga0 + t * VOCAB:ga0 + (t + 1) * VOCAB])
                        gsrc = gt[:]
                    else:
                        gsrc = gumb[:, ts30]
                    scores = sc.tile([B, VOCAB], f32, tag="scores", name=f"scores{t}")
                    nc.vector.tensor_add(scores[:], L[:], gsrc)

                    # ---- sample: argmax(scores) ----
                    mx8 = sc.tile([B, 8], f32, tag="mx8", name=f"mx8_{t}")
                    idx8 = sc.tile([B, 8], u32, tag="idx8", name=f"idx8_{t}")
                    nc.vector.max(mx8[:], scores[:])
                    nc.vector.max_index(idx8[:], mx8[:], scores[:])
                    samp = sc.tile([B, 1], f32, tag="samp", name=f"samp{t}")
                    nc.vector.tensor_copy(samp[:], idx8[:, 0:1])

                    # nie = (is_end == 0);  current = samp * nie  (PAD == 0)
                    nie = sc.tile([B, 1], f32, tag="nie", name=f"nie{t}")
                    nc.vector.tensor_scalar(nie[:], is_end[:], 0.0, None, ALU.is_equal)
                    nc.vector.tensor_mul(current[:], samp[:], nie[:])
                    nc.vector.tensor_copy(seq_f[:, t:t + 1], current[:])
                    # lengths += nie
                    nc.vector.tensor_add(lengths[:], lengths[:], nie[:])
                    # is_end |= (current == EOS)
                    eq2 = sc.tile([B, 1], f32, tag="eq2", name=f"eq2_{t}")
                    nc.vector.tensor_scalar(eq2[:], current[:], float(EOS), None,
                                            ALU.is_equal)
                    nc.vector.tensor_max(is_end[:], is_end[:], eq2[:])

            # ---- outputs (packed into one tensor) ----
            if not hybrid:
                nc.sync.dma_start(D_out[:, 0:T * VOCAB], out_log[:])
            nc.sync.dma_start(D_out[:, T * VOCAB:T * VOCAB + T], seq_f[:])
            nc.sync.dma_start(D_out[:, T * VOCAB + T:T * VOCAB + T + 1], lengths[:])

    split_multiwaits(nc)
    return nc


# ---------------------------------------------------------------- runner
def make_runner(nc, n_cores):
    import jax
    from concourse.bass2jax import (_bass_exec_p, install_neuronx_cc_hook,
                                    partition_id_tensor)
    from jax.sharding import Mesh, PartitionSpec
    from jax.experimental.shard_map import shard_map

    install_neuronx_cc_hook()
    partition_name = nc.partition_id_tensor.name if nc.partition_id_tensor else None
    in_names, out_names, out_avals, zero_outs = [], [], [], []
    for alloc in nc.m.functions[0].allocations:
        if not isinstance(alloc, mybir.MemoryLocationSet):
            continue
        name = alloc.memorylocations[0].name
        if alloc.kind == "ExternalInput":
            if name != partition_name:
                in_names.append(name)
        elif alloc.kind == "ExternalOutput":
            out_names.append(name)
            shape = tuple(alloc.tensor_shape)
            dtype = mybir.dt.np(alloc.dtype)
            out_avals.append(jax.core.ShapedArray(shape, dtype))
            zero_outs.append(np.zeros(shape, dtype))
    n_params = len(in_names)
    n_outs = len(out_avals)
    all_in_names = list(in_names) + list(out_names)
    if partition_name is not None:
        all_in_names.append(partition_name)

    def _body(*args):
        operands = list(args)
        if partition_name is not None:
            operands.append(partition_id_tensor())
        outs = _bass_exec_p.bind(
            *operands,
            out_avals=tuple(out_avals),
            in_names=tuple(all_in_names),
            out_names=tuple(out_names),
            lowering_input_output_aliases=(),
            sim_require_finite=True,
            sim_require_nnan=True,
            nc=nc,
        )
        return tuple(outs)

    devices = jax.devices()[:n_cores]
    mesh = Mesh(np.asarray(devices), ("core",))
    in_specs = (PartitionSpec("core"),) * (n_params + n_outs)
    out_specs = (PartitionSpec("core"),) * n_outs
    jf = jax.jit(
        shard_map(_body, mesh=mesh, in_specs=in_specs, out_specs=out_specs,
                  check_rep=False),
        keep_unused=True,
    )

    def _concat(in_maps):
        per_core = [[np.asarray(m[n]) for n in in_names] for m in in_maps]
        concat_in = [np.concatenate([per_core[c][i] for c in range(n_cores)], axis=0)
                     for i in range(n_params)]
        concat_zero = [np.concatenate([z] * n_cores, axis=0) for z in zero_outs]
        return concat_in, concat_zero

    def run(in_maps):
        concat_in, concat_zero = _concat(in_maps)
        outs = [np.asarray(o) for o in jf(*concat_in, *concat_zero)]
        res = []
        for c in range(n_cores):
            d = {}
            for i, n in enumerate(out_names):
                per = outs[i].shape[0] // n_cores
                d[n] = outs[i][c * per:(c + 1) * per]
            res.append(d)
        return res

    def time_exec(in_maps, ncalls=8):
        """Place inputs on device once, then time pure executable dispatch+run."""
        import time as _time
        from jax.sharding import NamedSharding
        sh = NamedSharding(mesh, PartitionSpec("core"))
        concat_in, concat_zero = _concat(in_maps)
        dev_in = [jax.device_put(a, sh) for a in concat_in]
        dev_zero = [jax.device_put(a, sh) for a in concat_zero]
        ts = []
        for _ in range(ncalls):
            t0 = _time.perf_counter()
            outs = jf(*dev_in, *dev_zero)
            for o in outs:
                o.block_until_ready()
            ts.append(_time.perf_counter() - t0)
        return ts

    run.time_exec = time_exec
    return run


# ---------------------------------------------------------------- host side
def _bf16(x):
    import jax.numpy as jnp
    return np.asarray(jnp.asarray(x, jnp.bfloat16).astype(jnp.float32))


def _gumbel_table(T):
    import jax
    with jax.default_device(jax.devices("cpu")[0]):
        keys = jax.random.split(jax.random.key(1), T)
        g = jax.random.gumbel(jax.vmap(lambda k: k)(keys), (T, B_FULL, VOCAB),
                              np.float32) if False else None
        import jax.numpy as jnp
        gs = [jax.random.gumbel(keys[t], (B_FULL, VOCAB), jnp.float32) for t in range(T)]
        return np.stack([np.asarray(x) for x in gs], axis=0)  # [T, B, V]


_CACHE = {}


def _get_runner(T):
    if T not in _CACHE:
        nc = build_nc(T)
        _CACHE[T] = (nc, make_runner(nc, N_CORES))
    return _CACHE[T]


def kernel(prevs, emb, w_ih_0, w_hh_0, b_ih_0, b_hh_0,
           w_ih_1, w_hh_1, b_ih_1, b_hh_1, w_out, b_out, max_len):
    T = int(max_len)
    prevs = np.asarray(prevs, np.int32)
    emb = np.asarray(emb, np.float32)
    w_ih_0 = np.asarray(w_ih_0, np.float32); w_hh_0 = np.asarray(w_hh_0, np.float32)
    b_ih_0 = np.asarray(b_ih_0, np.float32); b_hh_0 = np.asarray(b_hh_0, np.float32)
    w_ih_1 = np.asarray(w_ih_1, np.float32); w_hh_1 = np.asarray(w_hh_1, np.float32)
    b_ih_1 = np.asarray(b_ih_1, np.float32); b_hh_1 = np.asarray(b_hh_1, np.float32)
    w_out = np.asarray(w_out, np.float32); b_out = np.asarray(b_out, np.float32)

    # host-side weight prep, packed into one blob per core
    lay, ncols = _blob_layout(T)
    base = np.zeros((128, ncols), np.float32)

    def put(name, arr, rows=128):
        a, b = lay[name]
        base[:rows, a:b] = arr

    put("ident", np.eye(128, dtype=np.float32))
    w0e = np.empty((VOCAB + 1, H4), np.float32)
    w0e[:VOCAB] = (emb.astype(np.float64) @ w_ih_0.T.astype(np.float64)).astype(np.float32)
    w0e[VOCAB] = b_ih_0 + b_hh_0
    put("w0e", w0e, rows=VOCAB + 1)

    def ktmajor(wT):  # [512, X] -> [128, 4*X] with kt-major columns
        X = wT.shape[1]
        return wT.reshape(KT, 128, X).transpose(1, 0, 2).reshape(128, KT * X)

    put("whh0", ktmajor(np.ascontiguousarray(w_hh_0.T)))
    put("wih1", ktmajor(np.ascontiguousarray(w_ih_1.T)))
    put("whh1", ktmajor(np.ascontiguousarray(w_hh_1.T)))
    put("wout", ktmajor(np.ascontiguousarray(w_out.T)))
    b1 = (b_ih_1 + b_hh_1).astype(np.float32)
    a = _bf16(b1); r = b1 - a
    bb = _bf16(r); r2 = r - bb
    cc = _bf16(r2)
    put("b13", np.stack([a, bb, cc], axis=0).astype(np.float32), rows=3)
    put("bout", np.broadcast_to(b_out, (B, VOCAB)))
    put("iota", np.arange(VOCAB, dtype=np.float32).reshape(VOCAB, 1), rows=VOCAB)

    G = _gumbel_table(T)                                          # [T, B_FULL, V]
    ga, gb = lay["gumb"]
    pa, pb = lay["prev"]
    in_maps = []
    for c in range(N_CORES):
        bs = slice(c * B, (c + 1) * B)
        blob = base.copy()
        gc = np.transpose(G[:, bs, :], (1, 0, 2))                # [B, T, V]
        blob[:, ga:gb] = (gc + b_out[None, None, :]).reshape(B, T * VOCAB)
        blob[:, pa:pb] = prevs[bs].reshape(B, 1).astype(np.float32)
        in_maps.append({"blob": blob})

    global _LAST_IN_MAPS
    _LAST_IN_MAPS = in_maps
    _, run = _get_runner(T)
    res = run(in_maps)

    out = np.concatenate([r["out_o"] for r in res], axis=0)       # [B, T*V+T+1] f32
    logits = out[:, :T * VOCAB].reshape(B_FULL, T, VOCAB).astype(np.float32)
    seq = np.rint(out[:, T * VOCAB:T * VOCAB + T]).astype(np.int32)
    lengths = np.rint(out[:, T * VOCAB + T]).astype(np.int32)
    return seq, logits, lengths
